# revision 26
# baseline (speedup 1.0000x reference)
"""Trainium2 Bass kernel for 3D-PoPE multi-head self-attention.

Sharding: pure data-parallel over batch (B=8 -> 8 cores, one batch element
per core). Weights replicated. All rotary/cache gathers precomputed on host
(tiny tensors); all matmuls/softmax on device.

Per-core device computation (S=1024, D=1024, H=16, HD=64):
  - DMA interleaved hsT/wq chunks first so the q-projection starts ~2us in,
    then wk, wv, rot, wo in consumption order.
  - fused pipeline: q/k projection + softplus (exp+ln, single ACT table
    set) one block ahead of attention; v-projection overlaps the early
    attention blocks; ScalarE exp stream (the attention bottleneck) never
    waits on projection chains.
  - single shared PSUM ring (tag "ps", 3 bufs) for proj/scores/bcast/
    out-proj + a dedicated pv slot: 6+2 = 8 banks exactly.
  - per head: raw attention rows and the ones-augmented denominator row
    are evacuated to SBUF immediately (no normalization in the hot loop);
    normalization happens per head-pair, deferred one block: reciprocal,
    K=2 bf16 ones-matmul partition-broadcast, one in-place [128,S] mul.
  - out = attnT.T @ w_out.T
"""
import math

import numpy as np
import ml_dtypes

B, S, D, H = 8, 1024, 1024, 16
HD = D // H
DX = HD // 3
DY = HD // 3
DZ = HD - DX - DY
MX, MY, MZ = 32, 32, 8
BASE = 10000.0
TWO_PI = 2.0 * math.pi
BF16 = ml_dtypes.bfloat16

NT = S // 128          # 8 sequence tiles
ND = D // 128          # 8 contraction tiles
SCALE = 1.0 / math.sqrt(2.0 * HD)


def _host_prep(hidden_states, pos_xyz, w_qkv, w_out, phase_bias):
    """Host-side: transposes, dtype casts, rotary cache gather."""
    def cache(dim, maxp):
        inv = 1.0 / (BASE ** (np.arange(dim, dtype=np.float64) / dim))
        t = np.arange(maxp, dtype=np.float64)[:, None] * inv[None, :]
        return np.cos(t), np.sin(t)

    cx, sx = cache(DX, MX)
    cy, sy = cache(DY, MY)
    cz, sz = cache(DZ, MZ)
    pos = np.asarray(pos_xyz)
    px = np.clip(pos[..., 0], 0, MX - 1).astype(np.int64)
    py = np.clip(pos[..., 1], 0, MY - 1).astype(np.int64)
    pz = np.clip(pos[..., 2], 0, MZ - 1).astype(np.int64)
    cos_t = np.concatenate([cx[px], cy[py], cz[pz]], axis=-1)  # [B,S,HD] f64
    sin_t = np.concatenate([sx[px], sy[py], sz[pz]], axis=-1)
    bias = np.clip(np.asarray(phase_bias, np.float64), -TWO_PI, 0.0)
    cos_b = np.cos(bias)
    sin_b = np.sin(bias)
    cos_k = cos_t * cos_b - sin_t * sin_b
    sin_k = sin_t * cos_b + cos_t * sin_b

    def dup(x):
        # [B,S,HD] -> [B, 128, S] bf16 with rows 0:64 == rows 64:128
        xt = np.ascontiguousarray(x.transpose(0, 2, 1))  # [B, HD, S]
        return np.concatenate([xt, xt], axis=1).astype(BF16)

    hs = np.asarray(hidden_states, np.float32)
    hsT = np.ascontiguousarray(hs.transpose(0, 2, 1)).astype(BF16)  # [B, D, S]
    wqkvT = np.ascontiguousarray(np.asarray(w_qkv, np.float32).T).astype(BF16)
    woutT = np.ascontiguousarray(np.asarray(w_out, np.float32).T).astype(BF16)
    return hsT, wqkvT, woutT, dup(cos_t), dup(sin_t), dup(cos_k), dup(sin_k)


VARIANT = 'full'


def _emit(tc, nc, t_hsT, t_wqkvT, t_woutT, t_rot, t_out, variant='full'):
    import concourse.mybir as mybir

    dt = mybir.dt
    AF = mybir.ActivationFunctionType
    f32 = dt.float32
    bf = dt.bfloat16

    with (
        tc.tile_pool(name="main", bufs=1) as mp,
        tc.tile_pool(name="psum", bufs=1, space="PSUM") as pp,
    ):
        # ------------- loads (emission order = DMA priority) -------------
        hsT = mp.tile([128, ND * S], bf, tag="hsT", bufs=1)
        wq = mp.tile([128, ND * D], bf, tag="wq", bufs=1)
        wk = mp.tile([128, ND * D], bf, tag="wk", bufs=1)
        for i in range(ND):
            nc.sync.dma_start(
                hsT[:, i * S:(i + 1) * S], t_hsT[i * 128:(i + 1) * 128, :])
            nc.sync.dma_start(
                wq[:, i * D:(i + 1) * D], t_wqkvT[i * 128:(i + 1) * 128, 0:D])
            nc.sync.dma_start(
                wk[:, i * D:(i + 1) * D],
                t_wqkvT[i * 128:(i + 1) * 128, D:2 * D])

        def load_w(tag, col0):
            w = mp.tile([128, ND * D], bf, tag=tag, bufs=1, name=tag)
            for i in range(ND):
                nc.sync.dma_start(
                    w[:, i * D:(i + 1) * D],
                    t_wqkvT[i * 128:(i + 1) * 128, col0:col0 + D])
            return w

        wv = load_w("wv", 2 * D)
        rot = mp.tile([128, 4 * S], bf, tag="rot", bufs=1)
        nc.sync.dma_start(rot[:], t_rot[:])
        r_cq = rot[:, 0:S]
        r_sq = rot[:, S:2 * S]
        r_ck = rot[:, 2 * S:3 * S]
        r_sk = rot[:, 3 * S:4 * S]
        wo = mp.tile([128, ND * D], bf, tag="wo", bufs=1)
        for i in range(ND):
            nc.sync.dma_start(
                wo[:, i * D:(i + 1) * D], t_woutT[i * 128:(i + 1) * 128, :])

        vaug = mp.tile([128, NT * H * 65], bf, tag="vaug", bufs=1)
        va_r = vaug.rearrange("p (k h c) -> p k h c", k=NT, h=H)
        nc.vector.memset(va_r[:, :, :, 64:65], 1.0)

        ones_b = mp.tile([1, 64], bf, tag="ones_b", bufs=1)
        nc.vector.memset(ones_b[:], 1.0)

        attnU = mp.tile([128, 8 * S], bf, tag="attnU", bufs=1)

        # ---------------- building blocks ----------------
        def emit_proj(w, j):
            ps = pp.tile([128, S], f32, tag="ps", bufs=3, name="ps_qk")
            for di in range(ND):
                lhsT = w[:, di * D + j * 128:di * D + (j + 1) * 128]
                rhs = hsT[:, di * S:(di + 1) * S]
                nc.tensor.matmul(ps[:, 0:512], lhsT, rhs[:, 0:512],
                                 start=(di == 0), stop=(di == ND - 1))
                nc.tensor.matmul(ps[:, 512:1024], lhsT, rhs[:, 512:1024],
                                 start=(di == 0), stop=(di == ND - 1))
            return ps

        def emit_pope(ps, cosr, sinr, tag):
            # softplus(x) = ln(exp(x) + 1) — both fns in one ACT table set
            pse = mp.tile([128, S], bf, tag="pse", bufs=2, name="pse")
            nc.scalar.activation(pse[:], ps[:], AF.Exp)
            mu = mp.tile([128, S], bf, tag="mu_" + tag, bufs=2, name="mu")
            nc.scalar.activation(mu[:], pse[:], AF.Ln, bias=1.0)
            d2 = mp.tile([128, 2 * S], bf, tag=tag, bufs=2, name=tag)
            for hh in range(2):
                lo, hi = hh * 64, hh * 64 + 64
                dsl = d2[:, hh * S:(hh + 1) * S]
                nc.vector.tensor_mul(dsl[0:64, :], mu[lo:hi, :], cosr[lo:hi, :])
                nc.vector.tensor_mul(dsl[64:128, :], mu[lo:hi, :], sinr[lo:hi, :])
            return d2

        def emit_proj_sp(j):
            ps_q = emit_proj(wq, j)
            q2 = emit_pope(ps_q, r_cq, r_sq, "q2")
            ps_k = emit_proj(wk, j)
            k2 = emit_pope(ps_k, r_ck, r_sk, "k2")
            return q2, k2

        def emit_scores(q2, k2, kt):
            pss = pp.tile([128, S], f32, tag="ps", bufs=3, name="ps_s")
            k2s = k2[:, kt * 128:(kt + 1) * 128]
            nc.tensor.matmul(pss[:, 0:512], k2s, q2[:, 0:512],
                             start=True, stop=True)
            nc.tensor.matmul(pss[:, 512:1024], k2s, q2[:, 512:1024],
                             start=True, stop=True)
            return pss

        def emit_norm(j, rcbs):
            # normalize head pair (2j, 2j+1): broadcast 1/denom over 64
            # partitions each via K=1 bf16 ones-matmuls, one in-place mul
            pbc = pp.tile([128, S], f32, tag="ps", bufs=3, name="ps_bc")
            for hh in range(2):
                lo = hh * 64
                nc.tensor.matmul(pbc[lo:lo + 64, 0:512], ones_b[:],
                                 rcbs[hh][:, 0:512], start=True, stop=True)
                nc.tensor.matmul(pbc[lo:lo + 64, 512:1024], ones_b[:],
                                 rcbs[hh][:, 512:1024], start=True, stop=True)
            bc = mp.tile([128, S], bf, tag="bc", bufs=3, name="bc")
            nc.vector.tensor_copy(bc[:], pbc[:])
            dsl = attnU[:, j * S:(j + 1) * S]
            nc.vector.tensor_mul(dsl, dsl, bc[:])

        def emit_attention_head(h0, q2, k2, hh, norm_cb=None):
            h = h0 + hh
            q2h = q2[:, hh * S:(hh + 1) * S]
            k2h = k2[:, hh * S:(hh + 1) * S]
            pv = pp.tile([65, S], f32, tag="pvp", bufs=1, name="pv")
            pss = emit_scores(q2h, k2h, 0)
            pss_n = emit_scores(q2h, k2h, 1)
            for kt in range(NT):
                pss_nn = (emit_scores(q2h, k2h, kt + 2)
                          if kt + 2 < NT else None)
                pt = mp.tile([128, S], bf, tag="probsT", bufs=6, name="pt")
                nc.scalar.activation(pt[:], pss[:], AF.Exp, scale=SCALE)
                va = vaug[:, kt * H * 65 + h * 65:kt * H * 65 + (h + 1) * 65]
                nc.tensor.matmul(pv[:, 0:512], va, pt[:, 0:512],
                                 start=(kt == 0), stop=(kt == NT - 1))
                nc.tensor.matmul(pv[:, 512:1024], va, pt[:, 512:1024],
                                 start=(kt == 0), stop=(kt == NT - 1))
                pss, pss_n = pss_n, pss_nn
                if kt == 2 and norm_cb is not None:
                    norm_cb()
            # evacuate raw attention + 1/denominator; frees pv fast
            dsl = attnU[(h % 2) * 64:(h % 2) * 64 + 64,
                        (h // 2) * S:(h // 2 + 1) * S]
            nc.vector.tensor_copy(dsl, pv[0:64, :])
            rcb = mp.tile([1, S], bf, tag="rcb", bufs=4, name="rcb")
            with nc.allow_low_precision(reason="bf16 1/denom: 0.4% ok"):
                nc.vector.reciprocal(rcb[:], pv[64:65, :])
            return rcb

        # ---------------- emission schedule ----------------
        if variant == 'attn':
            nc.vector.memset(vaug[:, 0:NT * H * 65], 0.01)
            q2a = mp.tile([128, 2 * S], bf, tag="q2", bufs=2, name="q2m")
            k2a = mp.tile([128, 2 * S], bf, tag="k2", bufs=2, name="k2m")
            nc.vector.memset(q2a[:], 0.01)
            nc.vector.memset(k2a[:], 0.01)
            prev = None
            for j in range(ND):
                cb = (lambda p=prev: emit_norm(*p)) if prev is not None else None
                rcb0 = emit_attention_head(2 * j, q2a, k2a, 0, norm_cb=cb)
                rcb1 = emit_attention_head(2 * j, q2a, k2a, 1)
                prev = (j, [rcb0, rcb1])
            emit_norm(*prev)
            return
        q2a, k2a = emit_proj_sp(0)
        q2b, k2b = emit_proj_sp(1)

        # v projection (needs wv which lands after wq/wk; overlaps the
        # early attention blocks on the scheduler's backfill)
        for st in range(NT):
            ps = pp.tile([128, D], f32, tag="ps", bufs=3, name="ps_v")
            for di in range(ND):
                lhsT = hsT[:, di * S + st * 128:di * S + (st + 1) * 128]
                rhs = wv[:, di * D:(di + 1) * D]
                nc.tensor.matmul(ps[:, 0:512], lhsT, rhs[:, 0:512],
                                 start=(di == 0), stop=(di == ND - 1))
                nc.tensor.matmul(ps[:, 512:1024], lhsT, rhs[:, 512:1024],
                                 start=(di == 0), stop=(di == ND - 1))
            vr = va_r[:, st]
            nc.vector.tensor_copy(
                vr[:, :, 0:64], ps.rearrange("p (h c) -> p h c", c=64))

        if variant == 'proj':
            nc.vector.memset(attnU[:, 0:8 * S], 0.01)
            for j in range(2, ND):
                emit_pope(emit_proj(wq, j), r_cq, r_sq, "q2")
                emit_pope(emit_proj(wk, j), r_ck, r_sk, "k2")
        prev = None  # (j, rcbs) awaiting normalization
        for j in range(ND if variant != 'proj' else 0):
            # attention stream first (high priority: it paces ScalarE),
            # then the j+2 projection as low-priority PE filler that runs
            # in the exp-gated stall slices; split q/k around the heads so
            # each pvp ring slot is freed (by softplus-exp / pv
            # evacuation) well before its next user needs it
            q2b2 = k2b2 = None
            cb = (lambda p=prev: emit_norm(*p)) if prev is not None else None
            rcb0 = emit_attention_head(2 * j, q2a, k2a, 0, norm_cb=cb)
            if j + 2 < ND:
                q2b2 = emit_pope(emit_proj(wq, j + 2), r_cq, r_sq, "q2")
            rcb1 = emit_attention_head(2 * j, q2a, k2a, 1)
            if j + 2 < ND:
                k2b2 = emit_pope(emit_proj(wk, j + 2), r_ck, r_sk, "k2")
            prev = (j, [rcb0, rcb1])
            q2a, k2a = q2b, k2b
            q2b, k2b = q2b2, k2b2
        if prev is not None:
            emit_norm(*prev)

        # ---------------- output projection ----------------
        for st in range(NT):
            ps = pp.tile([128, D], f32, tag="ps", bufs=3, name="ps_o")
            for et in range(ND):
                lhsT = attnU[:, et * S + st * 128:et * S + (st + 1) * 128]
                rhs = wo[:, et * D:(et + 1) * D]
                nc.tensor.matmul(ps[:, 0:512], lhsT, rhs[:, 0:512],
                                 start=(et == 0), stop=(et == ND - 1))
                nc.tensor.matmul(ps[:, 512:1024], lhsT, rhs[:, 512:1024],
                                 start=(et == 0), stop=(et == ND - 1))
            ot = mp.tile([128, D], f32, tag="ot", bufs=2, name="ot")
            if st < NT - 1:
                nc.vector.tensor_copy(ot[:], ps[:])
                nc.sync.dma_start(t_out[st * 128:(st + 1) * 128, :], ot[:])
            else:
                # split the last tile so the final DMA tail is half-sized
                nc.vector.tensor_copy(ot[0:64, :], ps[0:64, :])
                nc.sync.dma_start(t_out[st * 128:st * 128 + 64, :], ot[0:64, :])
                nc.vector.tensor_copy(ot[64:128, :], ps[64:128, :])
                nc.sync.dma_start(t_out[st * 128 + 64:(st + 1) * 128, :],
                                  ot[64:128, :])


def build_bass(reps=1):
    import concourse.bass as bass  # noqa: F401
    import concourse.mybir as mybir
    import concourse.tile as tile
    from concourse import bacc

    # The only ACT functions this kernel uses are Exp and Ln, which share
    # the natural_log_exp_and_others table set. Restrict the table list so
    # the table-load inserter can't alternate between per-function sets
    # (each switch costs ~1.3us and the default greedy choice thrashes).
    if not getattr(bacc, "_act_tables_patched", False):
        _orig_tables = bacc.get_activation_tables

        def _only_shared(arch):
            # Keep every set at its canonical index (act_func_set_id is
            # positional), but make natural_log_exp_and_others the only
            # set offering Exp/Ln so the chooser can't thrash.
            t = _orig_tables(arch)
            AF = __import__("concourse.mybir", fromlist=["x"]).ActivationFunctionType
            out = {}
            for k, fns in t.items():
                if k == "natural_log_exp_and_others":
                    out[k] = fns
                else:
                    out[k] = {f for f in fns if f not in (AF.Exp, AF.Ln)}
            return out

        bacc.get_activation_tables = _only_shared
        bacc._act_tables_patched = True

    dt = mybir.dt
    nc = bacc.Bacc("TRN2", target_bir_lowering=False, debug=False)
    t_hsT = nc.dram_tensor("hsT", [D, S], dt.bfloat16, kind="ExternalInput").ap()
    t_wqkvT = nc.dram_tensor("wqkvT", [D, 3 * D], dt.bfloat16, kind="ExternalInput").ap()
    t_woutT = nc.dram_tensor("woutT", [D, D], dt.bfloat16, kind="ExternalInput").ap()
    t_rot = nc.dram_tensor("rot", [128, 4 * S], dt.bfloat16, kind="ExternalInput").ap()
    t_out = nc.dram_tensor("out", [S, D], dt.float32, kind="ExternalOutput").ap()
    with tile.TileContext(nc) as tc:
        if reps == 1:
            _emit(tc, nc, t_hsT, t_wqkvT, t_woutT, t_rot, t_out, variant=VARIANT)
        else:
            with tc.For_i(0, reps, 1):
                _emit(tc, nc, t_hsT, t_wqkvT, t_woutT, t_rot, t_out,
                      variant=VARIANT)
    nc.compile()
    return nc


def make_in_maps(hidden_states, pos_xyz, w_qkv, w_out, phase_bias):
    hsT, wqkvT, woutT, cq, sq, ck, sk = _host_prep(
        hidden_states, pos_xyz, w_qkv, w_out, phase_bias)
    rot = np.concatenate([cq, sq, ck, sk], axis=2)  # [B, 128, 4*S]
    return [
        {
            "hsT": np.ascontiguousarray(hsT[b]),
            "wqkvT": wqkvT,
            "woutT": woutT,
            "rot": np.ascontiguousarray(rot[b]),
        }
        for b in range(B)
    ]


def kernel(hidden_states, attention_mask, pos_xyz, w_qkv, w_out, phase_bias):
    from concourse.bass_utils import run_bass_kernel_spmd

    in_maps = make_in_maps(hidden_states, pos_xyz, w_qkv, w_out, phase_bias)
    nc = build_bass()
    res = run_bass_kernel_spmd(nc, in_maps, list(range(B)))
    out = np.stack([np.asarray(res.results[c]["out"]) for c in range(B)])
    return out.astype(np.float32)
